# revision 13
# baseline (speedup 1.0000x reference)
"""Trainium2 Bass kernel for a transformer decoder block (B=4, T=1024, C=1024, H=16).

Sharding: 8 cores = 4 batches x 2 sequence halves. Each core owns 512 query
rows of one batch and recomputes K/V for the full batch sequence (no
cross-core communication, single SPMD launch).

Per-core dataflow keeps activations in transposed layout [C_part, T_free] so
all matmuls consume weights in their natural [in, out] layout:
  - ln1 applied to host-transposed x^T -> ln1T (bf16)
  - q^T/k^T = w^T @ ln1T (heads pair-packed on partitions 0:64 / 64:128)
  - v in row layout [keys, head*65] with a trailing ones/mask column, so the
    P@V matmul also emits softmax denominators as row 64 of its PSUM output
  - S^T[keys, q] = k^T.T @ q^T (K=64 row-packed matmuls), exp on ScalarE with
    scale=1/sqrt(D); tgt mask applied multiplicatively to p (exp*mask), src
    mask folded into v rows
  - attn^T/sums -> scale by reciprocal sums -> O-projection lhsT directly
  - FFN: y1^T = w1.T @ ln3T, relu+bias fused, y2 = y1relu^T.T @ w2
"""

import functools
import os
import sys

for _p in ("/opt/trn_rl_repo", os.path.expanduser("~/.axon_site/_ro/trn_rl_repo")):
    if os.path.isdir(_p) and _p not in sys.path:
        sys.path.insert(0, _p)

import numpy as np
import ml_dtypes

import concourse.bass as bass
import concourse.tile as tile
from concourse import bacc, mybir
from concourse.bass import ts, ds
from concourse.bass_utils import run_bass_kernel_spmd
from concourse.masks import make_identity

B, T, C, H, D = 4, 1024, 1024, 16, 64
F = 4 * C
P = 128
R = 512  # query rows per core
NCORES = 8
EPS = 1e-5
CO = C // P   # 8 chunks of the channel dim
QT = R // P   # 4 query-row tiles
FO = F // P   # 32 FFN hidden tiles

F32 = mybir.dt.float32
BF16 = mybir.dt.bfloat16
AF = mybir.ActivationFunctionType
OP = mybir.AluOpType
BF16NP = ml_dtypes.bfloat16


def _stripe(ap):
    """[n*128] dram vector -> [128, n] (partition-striped, p inner)."""
    return ap.rearrange("(o p) -> p o", p=P)


def _r3(ap):
    """[(o p), n] dram matrix -> [p, o, n]."""
    return ap.rearrange("(o p) n -> p o n", p=P)


def _emit(tc, io):
    nc = tc.nc
    from contextlib import ExitStack

    with ExitStack() as ctx:
        const = ctx.enter_context(tc.tile_pool(name="const", bufs=1))
        resid = ctx.enter_context(tc.tile_pool(name="resid", bufs=1))
        psA = ctx.enter_context(tc.tile_pool(name="psA", bufs=2, space="PSUM"))
        psB = ctx.enter_context(tc.tile_pool(name="psB", bufs=1, space="PSUM"))
        lnT_pool = ctx.enter_context(tc.tile_pool(name="lnT", bufs=1))

        # ---- constants ----
        ident = const.tile([P, P], BF16, tag="ident")
        make_identity(nc, ident)
        ones_col = const.tile([P, 1], BF16, tag="ones_col")
        nc.vector.memset(ones_col, 1.0)

        lng = {}
        for nm in ("ln1_g", "ln1_b", "ln2_g", "ln2_b", "ln3_g", "ln3_b"):
            t = const.tile([P, CO], F32, tag=nm)
            nc.sync.dma_start(t[:], _stripe(io[nm]))
            lng[nm] = t
        b1s = const.tile([P, FO], F32, tag="b1s")
        nc.sync.dma_start(b1s[:], _stripe(io["ff_b1"]))
        bias_b = {}
        for nm in ("sa_bo_b", "ca_bo_b", "ff_b2_b"):
            t = const.tile([P, C], F32, tag=nm)
            nc.sync.dma_start(t[:], io[nm])
            bias_b[nm] = t
        srcm_s = const.tile([P, CO], F32, tag="srcm_s")
        nc.sync.dma_start(srcm_s[:], _stripe(io["srcm"]))

        # ---- residual stream (row layout [128, QT, C]) ----
        x0 = resid.tile([P, QT, C], F32, tag="resA")
        nc.sync.dma_start(x0[:], _r3(io["x0"]))
        x1 = resid.tile([P, QT, C], F32, tag="resB")

        # =========== LN1 over full batch, in transposed layout ===========
        ln1T = lnT_pool.tile([P, CO, T], BF16, tag="actT")
        if True:
            with tc.tile_pool(name="ln1", bufs=1) as ln1p:
                xT = ln1p.tile([P, CO, T], BF16, tag="xT")
                nc.sync.dma_start(xT[:], _r3(io["xT"]))
                sq = ln1p.tile([P, CO, T], BF16, tag="sq")
                nc.scalar.activation(sq[:], xT[:], AF.Square)

                rows = ln1p.tile([1, 6, T], F32, tag="rows")  # mu,msq,var,rs,a,b2
                mu, msq, var, rs_r, a_r, b2_r = (rows[:, i, :] for i in range(6))
                for th in range(2):
                    s1 = psB.tile([1, 512], F32, tag="stat")
                    for o in range(CO):
                        nc.tensor.matmul(s1[:], ones_col[:], xT[:, o, ts(th, 512)],
                                         start=(o == 0), stop=(o == CO - 1))
                    nc.vector.tensor_scalar_mul(mu[:, ts(th, 512)], s1[:], 1.0 / C)
                    s2 = psB.tile([1, 512], F32, tag="stat")
                    for o in range(CO):
                        nc.tensor.matmul(s2[:], ones_col[:], sq[:, o, ts(th, 512)],
                                         start=(o == 0), stop=(o == CO - 1))
                    nc.vector.tensor_scalar_mul(msq[:, ts(th, 512)], s2[:], 1.0 / C)
                nc.vector.tensor_mul(var[:], mu[:], mu[:])
                nc.vector.tensor_sub(var[:], msq[:], var[:])
                # rs = (var+eps)^-0.5 = exp(-0.5*ln(var+eps))
                nc.vector.tensor_scalar_add(var[:], var[:], EPS)
                nc.scalar.activation(rs_r[:], var[:], AF.Ln)
                nc.scalar.activation(rs_r[:], rs_r[:], AF.Exp, scale=-0.5)
                nc.vector.tensor_copy(a_r[:], rs_r[:])
                nc.vector.tensor_mul(b2_r[:], mu[:], rs_r[:])
                nc.vector.tensor_scalar_mul(b2_r[:], b2_r[:], -1.0)

                # broadcast a,b2 rows across partitions
                ab = ln1p.tile([P, 2, T], F32, tag="ab")
                nc.gpsimd.partition_broadcast(ab[:, 0, :], a_r)
                nc.gpsimd.partition_broadcast(ab[:, 1, :], b2_r)

                for o in range(CO):
                    tmp = ln1p.tile([P, T], F32, tag="tmp")
                    nc.vector.tensor_mul(tmp[:], xT[:, o, :], ab[:, 0, :])
                    nc.vector.tensor_add(tmp[:], tmp[:], ab[:, 1, :])
                    nc.vector.tensor_scalar(ln1T[:, o, :], tmp[:],
                                            lng["ln1_g"][:, o, None],
                                            lng["ln1_b"][:, o, None],
                                            OP.mult, OP.add)

        with tc.tile_pool(name="attnp", bufs=1) as attn, \
             tc.tile_pool(name="wstream", bufs=4) as wstream, \
             tc.tile_pool(name="wbig", bufs=2) as wbig, \
             tc.tile_pool(name="ptpool", bufs=2) as ptpool, \
             tc.tile_pool(name="ppool", bufs=2) as ppool, \
             tc.tile_pool(name="spool", bufs=2, space="PSUM") as spool:

            # =========== shared attention helpers ===========
            def proj_T(dst, w_dram, act, tn):
                """dst[:, j, :] = w[:, j*128:(j+1)*128].T @ act (accumulate cin)."""
                for j in range(CO):
                    wt = wstream.tile([P, CO, P], BF16, tag="wT")
                    nc.sync.dma_start(wt[:], _r3(w_dram)[:, :, ts(j, P)])
                    for t_ in range(tn // 512):
                        mm = psA.tile([P, 512], F32, tag="mm")
                        for o in range(CO):
                            nc.tensor.matmul(mm[:], wt[:, o, :],
                                             act[:, o, ts(t_, 512)],
                                             start=(o == 0), stop=(o == CO - 1))
                        nc.any.tensor_copy(dst[:, j, ts(t_, 512)], mm[:])

            def attention(xq_T, kv_T, wq, wk, wv, wo, bo_b, maskT, vscale,
                          out_x, in_x):
                """xq_T: [P, CO, R] bf16 (transposed queries); kv_T: [P, CO, T]."""
                qT = attn.tile([P, CO, R], BF16, tag="qT")
                kT = attn.tile([P, CO, T], BF16, tag="kT")
                vsb = attn.tile([P, CO, H, 65], BF16, tag="vsb")
                proj_T(qT, wq, xq_T, R)
                proj_T(kT, wk, kv_T, T)
                # v in row layout with trailing ones/mask column
                for ch in range(2):
                    wv_t = wbig.tile([P, CO, 512], BF16, tag="wv")
                    nc.sync.dma_start(wv_t[:], _r3(wv)[:, :, ts(ch, 512)])
                    for kt in range(CO):
                        mm = psA.tile([P, 512], F32, tag="mm")
                        for o in range(CO):
                            nc.tensor.matmul(mm[:], kv_T[:, o, ts(kt, P)],
                                             wv_t[:, o, :],
                                             start=(o == 0), stop=(o == CO - 1))
                        dstv = vsb[:, kt, ds(ch * 8, 8), 0:64]
                        srcv = mm.rearrange("p (h d) -> p h d", d=64)
                        if vscale is None:
                            nc.any.tensor_copy(dstv, srcv)
                        else:
                            nc.any.tensor_scalar_mul(dstv, srcv, vscale[:, kt, None])
                for kt in range(CO):
                    if vscale is None:
                        nc.vector.memset(vsb[:, kt, :, 64], 1.0)
                    else:
                        nc.vector.tensor_copy(vsb[:, kt, :, 64],
                                              vscale[:, kt, None].to_broadcast((P, H)))

                oT = attn.tile([P, CO, R], BF16, tag="oT")
                for h in range(H):
                    j, r = h // 2, 64 * (h % 2)
                    pT = ptpool.tile([P, CO, R], BF16, tag="pT")
                    for g in range(4):
                        sp = spool.tile([P, 2, 512], F32, tag="s")
                        for u in range(2):
                            kt = 2 * g + u
                            nc.tensor.matmul(sp[:, u, :],
                                             kT[r:r + 64, j, ts(kt, P)],
                                             qT[r:r + 64, j, :],
                                             start=True, stop=True)
                        nc.scalar.activation(pT[:, 2 * g:2 * g + 2, :], sp[:],
                                             AF.Exp, scale=float(1.0 / np.sqrt(D)))
                    if maskT is not None:
                        nc.vector.tensor_mul(pT[:], pT[:], maskT[:])
                    pv = psB.tile([65, 512], F32, tag="pv")
                    for kt in range(CO):
                        nc.tensor.matmul(pv[:], vsb[:, kt, h, :], pT[:, kt, :],
                                         start=(kt == 0), stop=(kt == CO - 1))
                    rec = ppool.tile([1, R], F32, tag="rec")
                    nc.vector.reciprocal(rec[:], pv[64:65, :])
                    recb = ppool.tile([64, R], F32, tag="recb")
                    nc.gpsimd.partition_broadcast(recb[:], rec[:])
                    nc.vector.tensor_mul(oT[r:r + 64, j, :], pv[0:64, :], recb[:])

                # O-projection + residual
                for ch in range(2):
                    wo_t = wbig.tile([P, CO, 512], BF16, tag="wv")
                    nc.sync.dma_start(wo_t[:], _r3(wo)[:, :, ts(ch, 512)])
                    for qt in range(QT):
                        mm = psA.tile([P, 512], F32, tag="mm")
                        for j in range(CO):
                            nc.tensor.matmul(mm[:], oT[:, j, ts(qt, P)],
                                             wo_t[:, j, :],
                                             start=(j == 0), stop=(j == CO - 1))
                        sl = ds(ch * 512, 512)
                        nc.vector.tensor_add(out_x[:, qt, sl], mm[:], in_x[:, qt, sl])
                        nc.vector.tensor_add(out_x[:, qt, sl], out_x[:, qt, sl],
                                             bo_b[:, sl])

            def ln_rows_to_T(src_x, g, b):
                """layernorm rows of src_x -> transposed+scaled [P, CO, R] bf16."""
                xhat = attn.tile([P, QT, C], BF16, tag="xhat")
                for qt in range(QT):
                    st6 = ppool.tile([P, 2, 6], F32, tag="st6")
                    for a_ in range(2):
                        nc.vector.bn_stats(st6[:, a_, :], src_x[:, qt, ts(a_, 512)])
                    mv = ppool.tile([P, 2], F32, tag="mv")
                    nc.vector.bn_aggr(mv[:], st6[:])
                    nmu = ppool.tile([P, 2], F32, tag="nmu")  # [-mean, rs]
                    nc.vector.tensor_scalar_mul(nmu[:, 0:1], mv[:, 0:1], -1.0)
                    nc.vector.tensor_scalar_add(nmu[:, 1:2], mv[:, 1:2], EPS)
                    nc.scalar.activation(nmu[:, 1:2], nmu[:, 1:2], AF.Ln)
                    nc.scalar.activation(nmu[:, 1:2], nmu[:, 1:2], AF.Exp, scale=-0.5)
                    nc.vector.tensor_scalar(xhat[:, qt, :], src_x[:, qt, :],
                                            nmu[:, 0:1], nmu[:, 1:2],
                                            OP.add, OP.mult)
                lnT = lnT_pool.tile([P, CO, R], BF16, tag="ln23T")
                for qt in range(QT):
                    for cb in range(CO):
                        tp = psA.tile([P, P], BF16, tag="mm")
                        nc.tensor.transpose(tp[:], xhat[:, qt, ts(cb, P)], ident[:])
                        nc.vector.tensor_scalar(lnT[:, cb, ts(qt, P)], tp[:],
                                                g[:, cb, None], b[:, cb, None],
                                                OP.mult, OP.add)
                return lnT

            # =========== self-attention ===========
            maskT = attn.tile([P, CO, R], BF16, tag="maskT")
            nc.sync.dma_start(maskT[:], _r3(io["maskT"]))
            ln1qT = ln_rows_to_T(x0, lng["ln1_g"], lng["ln1_b"])
            attention(ln1qT, ln1T, io["sa_wq"], io["sa_wk"], io["sa_wv"],
                      io["sa_wo"], bias_b["sa_bo_b"], maskT, None, x1, x0)

            # =========== cross-attention ===========
            encT = lnT_pool.tile([P, CO, T], BF16, tag="actT")
            nc.sync.dma_start(encT[:], _r3(io["encT"]))
            ln2T = ln_rows_to_T(x1, lng["ln2_g"], lng["ln2_b"])
            x2 = resid.tile([P, QT, C], F32, tag="resA")
            attention(ln2T, encT, io["ca_wq"], io["ca_wk"], io["ca_wv"],
                      io["ca_wo"], bias_b["ca_bo_b"], None, srcm_s, x2, x1)

            # =========== ln3 (still needs attn-phase scratch) ===========
            ln3T = ln_rows_to_T(x2, lng["ln3_g"], lng["ln3_b"])

        # =========== FFN ===========
        with tc.tile_pool(name="ffn", bufs=1) as ffn, \
             tc.tile_pool(name="ffnw", bufs=4) as ffnw:
            y1 = ffn.tile([P, FO, R], BF16, tag="y1")
            for f in range(FO):
                w1t = ffnw.tile([P, CO, P], BF16, tag="wT")
                nc.sync.dma_start(w1t[:], _r3(io["ff_w1"])[:, :, ts(f, P)])
                mm = psA.tile([P, 512], F32, tag="mm")
                for o in range(CO):
                    nc.tensor.matmul(mm[:], w1t[:, o, :], ln3T[:, o, :],
                                     start=(o == 0), stop=(o == CO - 1))
                nc.vector.tensor_scalar(y1[:, f, :], mm[:], b1s[:, f, None], 0.0,
                                        OP.add, OP.max)
            ost = ffn.tile([P, 2, 512], F32, tag="ost")
            for ch in range(2):
                w2t = ffn.tile([P, FO, 512], BF16, tag="w2t")
                nc.sync.dma_start(w2t[:], _r3(io["ff_w2"])[:, :, ts(ch, 512)])
                for qt in range(QT):
                    mm = psA.tile([P, 512], F32, tag="mm")
                    for f in range(FO):
                        nc.tensor.matmul(mm[:], y1[:, f, ts(qt, P)], w2t[:, f, :],
                                         start=(f == 0), stop=(f == FO - 1))
                    o_ = ost[:, ch, :]
                    sl = ds(ch * 512, 512)
                    nc.vector.tensor_add(o_, mm[:], x2[:, qt, sl])
                    nc.vector.tensor_add(o_, o_, bias_b["ff_b2_b"][:, sl])
                    nc.sync.dma_start(_r3(io["out"])[:, qt, sl], o_)


def build_program():
    nc = bacc.Bacc("TRN2", target_bir_lowering=False, debug=False,
                   enable_asserts=False, num_devices=NCORES)
    io = {}
    def inp(name, shape, dt):
        io[name] = nc.dram_tensor(name, list(shape), dt, kind="ExternalInput").ap()
    inp("xT", (C, T), BF16)
    inp("x0", (R, C), F32)
    inp("encT", (C, T), BF16)
    inp("maskT", (T, R), BF16)
    inp("srcm", (T,), F32)
    for nm in ("sa_wq", "sa_wk", "sa_wv", "sa_wo", "ca_wq", "ca_wk", "ca_wv", "ca_wo"):
        inp(nm, (C, C), BF16)
    inp("ff_w1", (C, F), BF16)
    inp("ff_w2", (F, C), BF16)
    for nm in ("ln1_g", "ln1_b", "ln2_g", "ln2_b", "ln3_g", "ln3_b"):
        inp(nm, (C,), F32)
    inp("ff_b1", (F,), F32)
    for nm in ("sa_bo_b", "ca_bo_b", "ff_b2_b"):
        inp(nm, (P, C), F32)
    io["out"] = nc.dram_tensor("out", [R, C], F32, kind="ExternalOutput").ap()

    with tile.TileContext(nc) as tc:
        _emit(tc, io)
    nc.compile()
    return nc


@functools.lru_cache(maxsize=None)
def _program():
    return build_program()


def _bf(a):
    return np.ascontiguousarray(a.astype(BF16NP))


def make_in_maps(inputs):
    x = np.asarray(inputs["x"], np.float32)
    enc = np.asarray(inputs["encoder_out"], np.float32)
    tgt = np.asarray(inputs["tgt_mask"]).reshape(T, T)
    src = np.asarray(inputs["src_mask"], np.float32).reshape(B, T)

    shared = {}
    for nm in ("sa_wq", "sa_wk", "sa_wv", "sa_wo", "ca_wq", "ca_wk", "ca_wv",
               "ca_wo", "ff_w1", "ff_w2"):
        shared[nm] = _bf(np.asarray(inputs[nm], np.float32))
    for nm in ("ln1_g", "ln1_b", "ln2_g", "ln2_b", "ln3_g", "ln3_b"):
        shared[nm] = np.ascontiguousarray(np.asarray(inputs[nm], np.float32))
    shared["ff_b1"] = np.ascontiguousarray(np.asarray(inputs["ff_b1"], np.float32))
    for nm, key in (("sa_bo_b", "sa_bo"), ("ca_bo_b", "ca_bo"), ("ff_b2_b", "ff_b2")):
        shared[nm] = np.ascontiguousarray(
            np.broadcast_to(np.asarray(inputs[key], np.float32)[None, :], (P, C)))

    in_maps = []
    for core in range(NCORES):
        b, hf = divmod(core, 2)
        rows = slice(hf * R, (hf + 1) * R)
        m = dict(shared)
        m["xT"] = _bf(x[b].T)
        m["x0"] = np.ascontiguousarray(x[b, rows])
        m["encT"] = _bf(enc[b].T)
        m["maskT"] = _bf(tgt[rows].T.astype(np.float32))
        m["srcm"] = np.ascontiguousarray(src[b])
        in_maps.append(m)
    return in_maps


def kernel(**inputs):
    in_maps = make_in_maps(inputs)
    res = run_bass_kernel_spmd(_program(), in_maps, list(range(NCORES)))
    out = np.empty((B, T, C), np.float32)
    for core in range(NCORES):
        b, hf = divmod(core, 2)
        out[b, hf * R:(hf + 1) * R] = res.results[core]["out"]
    return out


# revision 19
# speedup vs baseline: 1.0801x; 1.0801x over previous
"""Trainium2 Bass kernel for a transformer decoder block (B=4, T=1024, C=1024, H=16).

Sharding: 8 cores = 4 batches x 2 sequence halves. Each core owns 512 query
rows of one batch and recomputes K/V for the full batch sequence (no
cross-core communication, single SPMD launch).

Per-core dataflow keeps activations in transposed layout [C_part, T_free] so
all matmuls consume weights in their natural [in, out] layout:
  - ln1 applied to host-transposed x^T -> ln1T (bf16)
  - q^T/k^T = w^T @ ln1T (heads pair-packed on partitions 0:64 / 64:128)
  - v in row layout [keys, head*65] with a trailing ones/mask column, so the
    P@V matmul also emits softmax denominators as row 64 of its PSUM output
  - S^T[keys, q] = k^T.T @ q^T (K=64 row-packed matmuls), exp on ScalarE with
    scale=1/sqrt(D); tgt mask applied multiplicatively to p (exp*mask), src
    mask folded into v rows
  - attn^T/sums -> scale by reciprocal sums -> O-projection lhsT directly
  - FFN: y1^T = w1.T @ ln3T, relu+bias fused, y2 = y1relu^T.T @ w2
"""

import functools
import os
import sys

for _p in ("/opt/trn_rl_repo", os.path.expanduser("~/.axon_site/_ro/trn_rl_repo")):
    if os.path.isdir(_p) and _p not in sys.path:
        sys.path.insert(0, _p)

import numpy as np
import ml_dtypes

import concourse.bass as bass
import concourse.tile as tile
from concourse import bacc, mybir
from concourse.bass import ts, ds
from concourse.bass_utils import run_bass_kernel_spmd
from concourse.masks import make_identity

B, T, C, H, D = 4, 1024, 1024, 16, 64
F = 4 * C
P = 128
R = 512  # query rows per core
NCORES = 8
EPS = 1e-5
CO = C // P   # 8 chunks of the channel dim
QT = R // P   # 4 query-row tiles
FO = F // P   # 32 FFN hidden tiles

F32 = mybir.dt.float32
BF16 = mybir.dt.bfloat16
AF = mybir.ActivationFunctionType
OP = mybir.AluOpType
BF16NP = ml_dtypes.bfloat16
USE_NEWTON_RSQRT = os.environ.get("K_NEWTON", "0") == "1"
USE_APPROX_RECIP = os.environ.get("K_APPROX_RECIP", "1") == "1"



def _stripe(ap):
    """[n*128] dram vector -> [128, n] (partition-striped, p inner)."""
    return ap.rearrange("(o p) -> p o", p=P)


def _r3(ap):
    """[(o p), n] dram matrix -> [p, o, n]."""
    return ap.rearrange("(o p) n -> p o n", p=P)


def _emit(tc, io):
    nc = tc.nc
    from contextlib import ExitStack

    with ExitStack() as ctx:
        const = ctx.enter_context(tc.tile_pool(name="const", bufs=1))
        resid = ctx.enter_context(tc.tile_pool(name="resid", bufs=1))
        psA = ctx.enter_context(tc.tile_pool(name="psA", bufs=3, space="PSUM"))
        psB = ctx.enter_context(tc.tile_pool(name="psB", bufs=1, space="PSUM"))
        lnT_pool = ctx.enter_context(tc.tile_pool(name="lnT", bufs=1))

        # ---- constants ----
        ident = const.tile([P, P], BF16, tag="ident")
        make_identity(nc, ident)
        ones_col = const.tile([P, 1], BF16, tag="ones_col")
        nc.vector.memset(ones_col, 1.0)
        U32 = mybir.dt.uint32
        magic = const.tile([P, QT], U32, tag="magic")
        nc.vector._memset_packed(magic[:], 0x5F3759DF)

        def newton_rsqrt(dst, veps, scratch):
            """dst[P,n] = 1/sqrt(veps) on DVE only (bit-trick seed + 2 NR)."""
            n = veps.shape[-1]
            u = scratch[:, 0:n].bitcast(U32)
            nc.vector.tensor_scalar(u, veps.bitcast(U32), 1, None,
                                    OP.logical_shift_right)
            nc.vector.tensor_tensor(u, magic[:, 0:n].bitcast(U32), u, OP.subtract)
            y = scratch[:, 0:n]
            t = scratch[:, n:2 * n]
            for _ in range(3):
                nc.vector.tensor_mul(t, y, y)
                nc.vector.tensor_mul(t, t, veps)
                nc.vector.tensor_scalar(t, t, -0.5, 1.5, OP.mult, OP.add)
                nc.vector.tensor_mul(y, y, t)
            nc.vector.tensor_copy(dst, y)

        lng = {}
        for nm in ("ln1_g", "ln1_b", "ln2_g", "ln2_b", "ln3_g", "ln3_b"):
            t = const.tile([P, CO], F32, tag=nm)
            nc.sync.dma_start(t[:], _stripe(io[nm]))
            lng[nm] = t
        b1s = const.tile([P, FO], F32, tag="b1s")
        nc.sync.dma_start(b1s[:], _stripe(io["ff_b1"]))
        bias_b = {}
        for nm in ("sa_bo_b", "ca_bo_b", "ff_b2_b"):
            t = const.tile([P, C], F32, tag=nm)
            nc.sync.dma_start(t[:], io[nm])
            bias_b[nm] = t
        srcm_s = const.tile([P, CO], F32, tag="srcm_s")
        nc.sync.dma_start(srcm_s[:], _stripe(io["srcm"]))

        # ---- residual stream (row layout [128, QT, C]) ----
        x0 = resid.tile([P, QT, C], F32, tag="resA")
        nc.sync.dma_start(x0[:], _r3(io["x0"]))
        x1 = resid.tile([P, QT, C], F32, tag="resB")

        # =========== LN1 over full batch, in transposed layout ===========
        ln1T = lnT_pool.tile([P, CO, T], BF16, tag="actT")
        if True:
            with tc.tile_pool(name="ln1", bufs=1) as ln1p:
                xT = ln1p.tile([P, CO, T], BF16, tag="xT")
                nc.sync.dma_start(xT[:], _r3(io["xT"]))
                sq = ln1p.tile([P, CO, T], BF16, tag="sq")
                nc.vector.tensor_mul(sq[:], xT[:], xT[:])

                rows = ln1p.tile([1, 6, T], F32, tag="rows")  # mu,msq,var,rs,a,b2
                mu, msq, var, rs_r, a_r, b2_r = (rows[:, i, :] for i in range(6))
                for th in range(2):
                    s1 = psB.tile([1, 512], F32, tag="pv")
                    for o in range(CO):
                        nc.tensor.matmul(s1[:], ones_col[:], xT[:, o, ts(th, 512)],
                                         start=(o == 0), stop=(o == CO - 1))
                    nc.vector.tensor_scalar_mul(mu[:, ts(th, 512)], s1[:], 1.0 / C)
                    s2 = psB.tile([1, 512], F32, tag="pv")
                    for o in range(CO):
                        nc.tensor.matmul(s2[:], ones_col[:], sq[:, o, ts(th, 512)],
                                         start=(o == 0), stop=(o == CO - 1))
                    nc.vector.tensor_scalar_mul(msq[:, ts(th, 512)], s2[:], 1.0 / C)
                nc.vector.tensor_mul(var[:], mu[:], mu[:])
                nc.vector.tensor_sub(var[:], msq[:], var[:])
                # rs = (var+eps)^-0.5 = exp(-0.5*ln(var+eps))
                nc.vector.tensor_scalar_add(var[:], var[:], EPS)
                nc.scalar.activation(rs_r[:], var[:], AF.Ln)
                nc.scalar.activation(rs_r[:], rs_r[:], AF.Exp, scale=-0.5)
                nc.vector.tensor_copy(a_r[:], rs_r[:])
                nc.vector.tensor_mul(b2_r[:], mu[:], rs_r[:])
                nc.vector.tensor_scalar_mul(b2_r[:], b2_r[:], -1.0)

                # broadcast a,b2 rows across partitions
                ab = ln1p.tile([P, 2, T], F32, tag="ab")
                nc.gpsimd.partition_broadcast(ab[:, 0, :], a_r)
                nc.gpsimd.partition_broadcast(ab[:, 1, :], b2_r)

                for o in range(CO):
                    tmp = ln1p.tile([P, T], F32, tag="tmp")
                    nc.vector.tensor_mul(tmp[:], xT[:, o, :], ab[:, 0, :])
                    nc.vector.tensor_add(tmp[:], tmp[:], ab[:, 1, :])
                    nc.vector.tensor_scalar(ln1T[:, o, :], tmp[:],
                                            lng["ln1_g"][:, o, None],
                                            lng["ln1_b"][:, o, None],
                                            OP.mult, OP.add)

        with tc.tile_pool(name="attnp", bufs=1) as attn, \
             tc.tile_pool(name="wstream", bufs=4) as wstream, \
             tc.tile_pool(name="wbig", bufs=2) as wbig, \
             tc.tile_pool(name="ptpool", bufs=2) as ptpool, \
             tc.tile_pool(name="ppool", bufs=2) as ppool, \
             tc.tile_pool(name="spool", bufs=2, space="PSUM") as spool:

            # =========== shared attention helpers ===========
            def proj_T(dst, w_dram, act, tn):
                """dst[:, j, :] = w[:, j*128:(j+1)*128].T @ act (accumulate cin)."""
                for j in range(CO):
                    wt = wstream.tile([P, CO, P], BF16, tag="wT")
                    nc.sync.dma_start(wt[:], _r3(w_dram)[:, :, ts(j, P)])
                    for t_ in range(tn // 512):
                        mm = psA.tile([P, 512], F32, tag="mm")
                        for o in range(CO):
                            nc.tensor.matmul(mm[:], wt[:, o, :],
                                             act[:, o, ts(t_, 512)],
                                             start=(o == 0), stop=(o == CO - 1))
                        nc.any.tensor_copy(dst[:, j, ts(t_, 512)], mm[:])

            def attention(xq_T, kv_T, wq, wk, wv, wo, bo_b, maskT, vscale,
                          out_x, in_x):
                """xq_T: [P, CO, R] bf16 (transposed queries); kv_T: [P, CO, T]."""
                qT = attn.tile([P, CO, R], BF16, tag="qT")
                kT = attn.tile([P, CO, T], BF16, tag="kT")
                vsb = attn.tile([P, CO, H, 65], BF16, tag="vsb")
                proj_T(qT, wq, xq_T, R)
                proj_T(kT, wk, kv_T, T)
                # v in row layout with trailing ones/mask column
                for ch in range(2):
                    wv_t = wbig.tile([P, CO, 512], BF16, tag="wv")
                    nc.sync.dma_start(wv_t[:], _r3(wv)[:, :, ts(ch, 512)])
                    for kt in range(CO):
                        mm = psA.tile([P, 512], F32, tag="mm")
                        for o in range(CO):
                            nc.tensor.matmul(mm[:], kv_T[:, o, ts(kt, P)],
                                             wv_t[:, o, :],
                                             start=(o == 0), stop=(o == CO - 1))
                        dstv = vsb[:, kt, ds(ch * 8, 8), 0:64]
                        srcv = mm.rearrange("p (h d) -> p h d", d=64)
                        if vscale is None:
                            nc.any.tensor_copy(dstv, srcv)
                        else:
                            nc.any.tensor_scalar_mul(dstv, srcv, vscale[:, kt, None])
                for kt in range(CO):
                    if vscale is None:
                        nc.vector.memset(vsb[:, kt, :, 64], 1.0)
                    else:
                        nc.vector.tensor_copy(vsb[:, kt, :, 64],
                                              vscale[:, kt, None].to_broadcast((P, H)))

                oT = attn.tile([P, CO, R], BF16, tag="oT")
                sums = attn.tile([H, R], F32, tag="sums")
                sumsr = attn.tile([H, R], F32, tag="sumsr")
                for h in range(H):
                    j, r = h // 2, 64 * (h % 2)
                    pT = ptpool.tile([P, CO, R], BF16, tag="pT")
                    for g in range(4):
                        sp = spool.tile([P, 2, 512], F32, tag="s")
                        for u in range(2):
                            kt = 2 * g + u
                            nc.tensor.matmul(sp[:, u, :],
                                             kT[r:r + 64, j, ts(kt, P)],
                                             qT[r:r + 64, j, :],
                                             start=True, stop=True)
                        nc.scalar.activation(pT[:, 2 * g:2 * g + 2, :], sp[:],
                                             AF.Exp, scale=float(1.0 / np.sqrt(D)))
                    if maskT is not None:
                        nc.vector.tensor_mul(pT[:], pT[:], maskT[:])
                    pv = psB.tile([65, 512], F32, tag="pv")
                    for kt in range(CO):
                        nc.tensor.matmul(pv[:], vsb[:, kt, h, :], pT[:, kt, :],
                                         start=(kt == 0), stop=(kt == CO - 1))
                    # stash unnormalized attn + sums, freeing the PSUM bank
                    nc.any.tensor_copy(oT[r:r + 64, j, :], pv[0:64, :])
                    rec = ppool.tile([1, R], F32, tag="rec")
                    nc.vector.tensor_copy(rec[:], pv[64:65, :])
                    nc.sync.dma_start(sums[h:h + 1, :], rec[:])
                nc.vector.reciprocal(sumsr[:], sums[:])
                for h in range(H):
                    j, r = h // 2, 64 * (h % 2)
                    rh = ppool.tile([1, R], F32, tag="rec2")
                    nc.sync.dma_start(rh[:], sumsr[h:h + 1, :])
                    recb = ppool.tile([P, R], F32, tag="recb")
                    nc.gpsimd.partition_broadcast(recb[:], rh[:])
                    nc.vector.tensor_mul(oT[r:r + 64, j, :], oT[r:r + 64, j, :],
                                         recb[r:r + 64, :])

                # O-projection + residual (qt-outer so x rows complete early)
                wo_ts = []
                for ch in range(2):
                    wo_t = wbig.tile([P, CO, 512], BF16, tag="wv")
                    nc.sync.dma_start(wo_t[:], _r3(wo)[:, :, ts(ch, 512)])
                    wo_ts.append(wo_t)
                for qt in range(QT):
                    for ch in range(2):
                        mm = psA.tile([P, 512], F32, tag="mm")
                        for j in range(CO):
                            nc.tensor.matmul(mm[:], oT[:, j, ts(qt, P)],
                                             wo_ts[ch][:, j, :],
                                             start=(j == 0), stop=(j == CO - 1))
                        sl = ds(ch * 512, 512)
                        nc.vector.tensor_add(out_x[:, qt, sl], mm[:], in_x[:, qt, sl])
                        nc.vector.tensor_add(out_x[:, qt, sl], out_x[:, qt, sl],
                                             bo_b[:, sl])

            def ln_rows_to_T(src_x, g, b):
                """layernorm rows of src_x -> transposed+scaled [P, CO, R] bf16."""
                xhat = attn.tile([P, QT, C], BF16, tag="xhat")
                mvq = ppool.tile([P, QT, 2], F32, tag="mvq")  # per-qt [mean, var]
                for qt in range(QT):
                    st6 = ppool.tile([P, 2, 6], F32, tag="st6")
                    for a_ in range(2):
                        nc.vector.bn_stats(st6[:, a_, :], src_x[:, qt, ts(a_, 512)])
                    nc.vector.bn_aggr(mvq[:, qt, :], st6[:])
                nm = ppool.tile([P, 2, QT], F32, tag="nm")  # [-mean | rs] per qt
                scr = ppool.tile([P, 2 * QT], F32, tag="scr")
                veps = ppool.tile([P, QT], F32, tag="veps")
                nc.vector.tensor_scalar(nm[:, 0, :], mvq[:, :, 0], -1.0, None, OP.mult)
                nc.vector.tensor_scalar(veps[:], mvq[:, :, 1], EPS, None, OP.add)
                if USE_NEWTON_RSQRT:
                    newton_rsqrt(nm[:, 1, :], veps[:], scr)
                else:
                    nc.scalar.activation(nm[:, 1, :], veps[:], AF.Ln)
                    nc.scalar.activation(nm[:, 1, :], nm[:, 1, :], AF.Exp, scale=-0.5)
                for qt in range(QT):
                    nc.vector.tensor_scalar(xhat[:, qt, :], src_x[:, qt, :],
                                            nm[:, 0, qt, None], nm[:, 1, qt, None],
                                            OP.add, OP.mult)
                lnT = lnT_pool.tile([P, CO, R], BF16, tag="ln23T")
                for qt in range(QT):
                    for cb in range(CO):
                        tp = psA.tile([P, P], BF16, tag="mm")
                        nc.tensor.transpose(tp[:], xhat[:, qt, ts(cb, P)], ident[:])
                        nc.vector.tensor_scalar(lnT[:, cb, ts(qt, P)], tp[:],
                                                g[:, cb, None], b[:, cb, None],
                                                OP.mult, OP.add)
                return lnT

            # =========== self-attention ===========
            maskT = attn.tile([P, CO, R], BF16, tag="maskT")
            nc.sync.dma_start(maskT[:], _r3(io["maskT"]))
            ln1qT = ln_rows_to_T(x0, lng["ln1_g"], lng["ln1_b"])
            attention(ln1qT, ln1T, io["sa_wq"], io["sa_wk"], io["sa_wv"],
                      io["sa_wo"], bias_b["sa_bo_b"], maskT, None, x1, x0)

            # =========== cross-attention ===========
            encT = lnT_pool.tile([P, CO, T], BF16, tag="actT")
            nc.sync.dma_start(encT[:], _r3(io["encT"]))
            ln2T = ln_rows_to_T(x1, lng["ln2_g"], lng["ln2_b"])
            x2 = resid.tile([P, QT, C], F32, tag="resA")
            attention(ln2T, encT, io["ca_wq"], io["ca_wk"], io["ca_wv"],
                      io["ca_wo"], bias_b["ca_bo_b"], None, srcm_s, x2, x1)

            # =========== ln3 (still needs attn-phase scratch) ===========
            ln3T = ln_rows_to_T(x2, lng["ln3_g"], lng["ln3_b"])

        # =========== FFN ===========
        with tc.tile_pool(name="ffn", bufs=1) as ffn, \
             tc.tile_pool(name="ffnw", bufs=4) as ffnw:
            y1 = ffn.tile([P, FO, R], BF16, tag="y1")
            for f in range(FO):
                w1t = ffnw.tile([P, CO, P], BF16, tag="wT")
                nc.sync.dma_start(w1t[:], _r3(io["ff_w1"])[:, :, ts(f, P)])
                mm = psA.tile([P, 512], F32, tag="mm")
                for o in range(CO):
                    nc.tensor.matmul(mm[:], w1t[:, o, :], ln3T[:, o, :],
                                     start=(o == 0), stop=(o == CO - 1))
                nc.vector.tensor_scalar(y1[:, f, :], mm[:], b1s[:, f, None], 0.0,
                                        OP.add, OP.max)
            ost = ffn.tile([P, 2, 512], F32, tag="ost")
            for ch in range(2):
                w2t = ffn.tile([P, FO, 512], BF16, tag="w2t")
                nc.sync.dma_start(w2t[:], _r3(io["ff_w2"])[:, :, ts(ch, 512)])
                for qt in range(QT):
                    mm = psA.tile([P, 512], F32, tag="mm")
                    for f in range(FO):
                        nc.tensor.matmul(mm[:], y1[:, f, ts(qt, P)], w2t[:, f, :],
                                         start=(f == 0), stop=(f == FO - 1))
                    o_ = ost[:, ch, :]
                    sl = ds(ch * 512, 512)
                    nc.vector.tensor_add(o_, mm[:], x2[:, qt, sl])
                    nc.vector.tensor_add(o_, o_, bias_b["ff_b2_b"][:, sl])
                    nc.sync.dma_start(_r3(io["out"])[:, qt, sl], o_)


def build_program():
    nc = bacc.Bacc("TRN2", target_bir_lowering=False, debug=False,
                   enable_asserts=False, num_devices=NCORES)
    io = {}
    def inp(name, shape, dt):
        io[name] = nc.dram_tensor(name, list(shape), dt, kind="ExternalInput").ap()
    inp("xT", (C, T), BF16)
    inp("x0", (R, C), F32)
    inp("encT", (C, T), BF16)
    inp("maskT", (T, R), BF16)
    inp("srcm", (T,), F32)
    for nm in ("sa_wq", "sa_wk", "sa_wv", "sa_wo", "ca_wq", "ca_wk", "ca_wv", "ca_wo"):
        inp(nm, (C, C), BF16)
    inp("ff_w1", (C, F), BF16)
    inp("ff_w2", (F, C), BF16)
    for nm in ("ln1_g", "ln1_b", "ln2_g", "ln2_b", "ln3_g", "ln3_b"):
        inp(nm, (C,), F32)
    inp("ff_b1", (F,), F32)
    for nm in ("sa_bo_b", "ca_bo_b", "ff_b2_b"):
        inp(nm, (P, C), F32)
    io["out"] = nc.dram_tensor("out", [R, C], F32, kind="ExternalOutput").ap()

    with tile.TileContext(nc) as tc:
        _emit(tc, io)
    nc.compile()
    return nc


@functools.lru_cache(maxsize=None)
def _program():
    return build_program()


def _bf(a):
    return np.ascontiguousarray(a.astype(BF16NP))


def make_in_maps(inputs):
    x = np.asarray(inputs["x"], np.float32)
    enc = np.asarray(inputs["encoder_out"], np.float32)
    tgt = np.asarray(inputs["tgt_mask"]).reshape(T, T)
    src = np.asarray(inputs["src_mask"], np.float32).reshape(B, T)

    shared = {}
    for nm in ("sa_wq", "sa_wk", "sa_wv", "sa_wo", "ca_wq", "ca_wk", "ca_wv",
               "ca_wo", "ff_w1", "ff_w2"):
        shared[nm] = _bf(np.asarray(inputs[nm], np.float32))
    for nm in ("ln1_g", "ln1_b", "ln2_g", "ln2_b", "ln3_g", "ln3_b"):
        shared[nm] = np.ascontiguousarray(np.asarray(inputs[nm], np.float32))
    shared["ff_b1"] = np.ascontiguousarray(np.asarray(inputs["ff_b1"], np.float32))
    for nm, key in (("sa_bo_b", "sa_bo"), ("ca_bo_b", "ca_bo"), ("ff_b2_b", "ff_b2")):
        shared[nm] = np.ascontiguousarray(
            np.broadcast_to(np.asarray(inputs[key], np.float32)[None, :], (P, C)))

    in_maps = []
    for core in range(NCORES):
        b, hf = divmod(core, 2)
        rows = slice(hf * R, (hf + 1) * R)
        m = dict(shared)
        m["xT"] = _bf(x[b].T)
        m["x0"] = np.ascontiguousarray(x[b, rows])
        m["encT"] = _bf(enc[b].T)
        m["maskT"] = _bf(tgt[rows].T.astype(np.float32))
        m["srcm"] = np.ascontiguousarray(src[b])
        in_maps.append(m)
    return in_maps


def kernel(**inputs):
    in_maps = make_in_maps(inputs)
    res = run_bass_kernel_spmd(_program(), in_maps, list(range(NCORES)))
    out = np.empty((B, T, C), np.float32)
    for core in range(NCORES):
        b, hf = divmod(core, 2)
        out[b, hf * R:(hf + 1) * R] = res.results[core]["out"]
    return out
